# revision 12
# baseline (speedup 1.0000x reference)
"""MoE layer (8 experts, top-2, SwiGLU) on 8 Trainium2 NeuronCores.

Strategy (expert-parallel, per the sharding hint):
  - Host computes the tiny gate (0.07% of FLOPs) in float64 — this is the
    routing/dispatch metadata plus the load-balance loss scalar.
  - Tokens are dispatched to their top-2 experts; expert e's token set is
    gathered, transposed to [DIM, C] and shipped to core e.
  - Each core runs the full SwiGLU FFN for its expert in float32r (full-rate
    fp32 on the PE array) over its gathered tokens:
      phase A: hT = w1^T xT ; gT = silu(w2^T xT) ; hgT = hT*gT -> DRAM stage
      phase B: yT = w3^T hgT, scaled per-token by the combine weight.
  - Host scatter-adds the two weighted expert contributions per token.

All activations stay transposed ([feature, token]) so every matmul uses the
weights in their natural layout and no on-device transposes are needed.
"""

import os
import sys

for _p in ("/opt/trn_rl_repo", "/root/.axon_site/_ro/trn_rl_repo"):
    if os.path.isdir(_p) and _p not in sys.path:
        sys.path.insert(0, _p)

import numpy as np

DIM = 1024
HID = 2048
E = 8
TOPK = 2
INV_SQRT2 = 1.0 / 1.41421356237
P = 128
KO_D = DIM // P  # 8 k-subtiles for DIM contraction
KO_H = HID // P  # 16 k-subtiles for HID contraction
JCH = 4  # hidden chunks in phase A (HID/JCH = 512 wide)
HC = HID // JCH  # 512
MI_N = HC // P  # 4 psum m-subtiles per hidden chunk

_cache = {}


def _build_nc(blocks, C, use_b12):
    import concourse.mybir as mybir
    import concourse.tile as tile
    from concourse import bacc

    R = mybir.dt.float32r
    F = mybir.dt.float32

    nc = bacc.Bacc("TRN2", target_bir_lowering=False, debug=False, num_devices=E)

    xT = nc.dram_tensor("xT", [DIM, C], R, kind="ExternalInput")
    w1 = nc.dram_tensor("w1", [DIM, HID], R, kind="ExternalInput")
    w2 = nc.dram_tensor("w2", [DIM, HID], R, kind="ExternalInput")
    w3 = nc.dram_tensor("w3", [HID, DIM], R, kind="ExternalInput")
    wtok = nc.dram_tensor("wtok", [P, C], F, kind="ExternalInput")
    if use_b12:
        b1d = nc.dram_tensor("b1", [HID], F, kind="ExternalInput")
        b2d = nc.dram_tensor("b2", [HID], F, kind="ExternalInput")
    y = nc.dram_tensor("y", [DIM, C], F, kind="ExternalOutput")

    xT_t = xT.ap().rearrange("(ko p) c -> p ko c", p=P)
    w1_t = w1.ap().rearrange("(ko p) h -> p ko h", p=P)
    w2_t = w2.ap().rearrange("(ko p) h -> p ko h", p=P)
    w3_t = w3.ap().rearrange("(ko p) d -> p ko d", p=P)
    y_t = y.ap().rearrange("(mo p) c -> p mo c", p=P)

    W3PRE = 8  # w3 k-chunks prefetched into spare SBUF during phase A

    with tile.TileContext(nc) as tc:
        with (
            tc.tile_pool(name="dram", bufs=1, space="DRAM") as drampool,
            tc.tile_pool(name="w3pre", bufs=1) as w3pre,
        ):
            hg_stage = drampool.tile([P, KO_H, C], R)

            w3_sb = [None] * KO_H
            wtok_sb = None

            # ---------------- Phase A: hgT = (w1^T x^T) * silu(w2^T x^T)
            with (
                tc.tile_pool(name="wA", bufs=1) as wA,
                tc.tile_pool(name="xA", bufs=2) as xA,
                tc.tile_pool(name="workA", bufs=2) as workA,
                tc.tile_pool(name="psA", bufs=1, space="PSUM") as psA,
            ):
                def load_xblock(c0, bs):
                    tiles = []
                    for ko in range(KO_D):
                        t = xA.tile([P, 512], R, tag=f"xb{ko}", name=f"xb{ko}")
                        nc.sync.dma_start(t[:, :bs], xT_t[:, ko, c0 : c0 + bs])
                        tiles.append(t)
                    return tiles

                # Block-0 activations first so the first matmuls start early.
                xb0 = load_xblock(*blocks[0])
                xb1 = None

                # Weights emitted in need order: j0, j1, (block-1 x), j2, j3.
                w1_sb = [[None] * KO_D for _ in range(JCH)]
                w2_sb = [[None] * KO_D for _ in range(JCH)]
                for j in range(JCH):
                    for ko in range(KO_D):
                        t1 = wA.tile([P, HC], R, tag=f"w1j{j}k{ko}", name=f"w1j{j}k{ko}")
                        nc.sync.dma_start(t1[:], w1_t[:, ko, j * HC : (j + 1) * HC])
                        t2 = wA.tile([P, HC], R, tag=f"w2j{j}k{ko}", name=f"w2j{j}k{ko}")
                        nc.sync.dma_start(t2[:], w2_t[:, ko, j * HC : (j + 1) * HC])
                        w1_sb[j][ko] = t1
                        w2_sb[j][ko] = t2
                    if j == 1 and len(blocks) > 1:
                        xb1 = load_xblock(*blocks[1])

                if use_b12:
                    b1_sb = wA.tile([P, KO_H], F, tag="b1")
                    b2_sb = wA.tile([P, KO_H], F, tag="b2")
                    nc.sync.dma_start(
                        b1_sb[:], b1d.ap().rearrange("(ko p) -> p ko", p=P)
                    )
                    nc.sync.dma_start(
                        b2_sb[:], b2d.ap().rearrange("(ko p) -> p ko", p=P)
                    )

                for bi_, (c0, bs) in enumerate(blocks):
                    if bi_ == 0:
                        xb = xb0
                    elif bi_ == 1:
                        xb = xb1
                    else:
                        xb = load_xblock(c0, bs)
                    for j in range(JCH):
                        for mi in range(MI_N):
                            hs = slice(mi * P, (mi + 1) * P)
                            h_ps = psA.tile([P, 512], F, tag=f"h{mi}")
                            g_ps = psA.tile([P, 512], F, tag=f"g{mi}")
                            for ki in range(KO_D):
                                nc.tensor.matmul(
                                    h_ps[:, :bs],
                                    w1_sb[j][ki][:, hs],
                                    xb[ki][:, :bs],
                                    start=(ki == 0),
                                    stop=(ki == KO_D - 1),
                                )
                            for ki in range(KO_D):
                                nc.tensor.matmul(
                                    g_ps[:, :bs],
                                    w2_sb[j][ki][:, hs],
                                    xb[ki][:, :bs],
                                    start=(ki == 0),
                                    stop=(ki == KO_D - 1),
                                )
                            hidx = j * MI_N + mi
                            gact = workA.tile([P, 512], F, tag="gact")
                            if use_b12:
                                nc.scalar.activation(
                                    gact[:, :bs],
                                    g_ps[:, :bs],
                                    mybir.ActivationFunctionType.Silu,
                                    bias=b2_sb[:, hidx : hidx + 1],
                                )
                                nc.vector.tensor_scalar_add(
                                    h_ps[:, :bs],
                                    h_ps[:, :bs],
                                    b1_sb[:, hidx : hidx + 1],
                                )
                            else:
                                nc.scalar.activation(
                                    gact[:, :bs],
                                    g_ps[:, :bs],
                                    mybir.ActivationFunctionType.Silu,
                                )
                            hg = workA.tile([P, 512], R, tag="hg")
                            nc.vector.tensor_mul(
                                hg[:, :bs], h_ps[:, :bs], gact[:, :bs]
                            )
                            nc.sync.dma_start(
                                hg_stage[:, hidx, c0 : c0 + bs], hg[:, :bs]
                            )

                # Prefetch half of w3 + wtok into spare SBUF while phase A
                # drains (low priority: emitted last).
                for ko in range(W3PRE):
                    t = w3pre.tile([P, DIM], R, tag=f"w3k{ko}", name=f"w3k{ko}")
                    nc.sync.dma_start(t[:], w3_t[:, ko, :])
                    w3_sb[ko] = t

            # ---------------- Phase B: yT = w3^T hgT, scaled by wtok
            with (
                tc.tile_pool(name="wB", bufs=1) as wB,
                tc.tile_pool(name="hgB", bufs=3) as hgB,
                tc.tile_pool(name="outB", bufs=4) as outB,
                tc.tile_pool(name="psB", bufs=1, space="PSUM") as psB,
            ):
                wtok_sb = wB.tile([P, C], F, tag="wtok")
                nc.sync.dma_start(wtok_sb[:], wtok.ap())
                for ko in range(W3PRE, KO_H):
                    t = wB.tile([P, DIM], R, tag=f"w3k{ko}", name=f"w3k{ko}")
                    nc.sync.dma_start(t[:], w3_t[:, ko, :])
                    w3_sb[ko] = t

                MO_N = DIM // P
                for c0, bs in blocks:
                    hgb = []
                    for ko in range(KO_H):
                        t = hgB.tile([P, 512], R, tag=f"hgb{ko}", name=f"hgb{ko}")
                        nc.sync.dma_start(
                            t[:, :bs], hg_stage[:, ko, c0 : c0 + bs]
                        )
                        hgb.append(t)
                    # ki-outer: all 8 output banks accumulate in parallel, so
                    # compute starts as soon as one w3 chunk + one hg sliver
                    # are resident.
                    y_ps = [
                        psB.tile([P, 512], F, tag=f"y{mo}", name=f"y{mo}")
                        for mo in range(MO_N)
                    ]
                    for ki in range(KO_H):
                        for mo in range(MO_N):
                            nc.tensor.matmul(
                                y_ps[mo][:, :bs],
                                w3_sb[ki][:, mo * P : (mo + 1) * P],
                                hgb[ki][:, :bs],
                                start=(ki == 0),
                                stop=(ki == KO_H - 1),
                            )
                    for mo in range(MO_N):
                        y_sb = outB.tile([P, 512], F, tag="ysb")
                        nc.vector.tensor_mul(
                            y_sb[:, :bs], y_ps[mo][:, :bs], wtok_sb[:, c0 : c0 + bs]
                        )
                        nc.sync.dma_start(y_t[:, mo, c0 : c0 + bs], y_sb[:, :bs])

    nc.compile()
    return nc


def _get_nc(blocks, C, use_b12):
    key = (tuple(blocks), C, use_b12)
    if key not in _cache:
        _cache[key] = _build_nc(blocks, C, use_b12)
    return _cache[key]


def kernel(x, gate_w, gate_b, w1, b1, w2, b2, w3, b3):
    from concourse.bass_utils import run_bass_kernel_spmd

    x = np.asarray(x, dtype=np.float32)
    gate_w = np.asarray(gate_w, dtype=np.float32)
    gate_b = np.asarray(gate_b, dtype=np.float32)
    w1 = np.asarray(w1, dtype=np.float32)
    b1 = np.asarray(b1, dtype=np.float32)
    w2 = np.asarray(w2, dtype=np.float32)
    b2 = np.asarray(b2, dtype=np.float32)
    w3 = np.asarray(w3, dtype=np.float32)
    b3 = np.asarray(b3, dtype=np.float32)

    B, S, D = x.shape
    N = B * S
    xf = x.reshape(N, D)

    # ---- Gating / routing metadata (float64 so top-k decisions are exact)
    logits = xf.astype(np.float64) @ gate_w.astype(np.float64) + gate_b
    logits -= logits.max(axis=-1, keepdims=True)
    sc = np.exp(logits)
    sc /= sc.sum(axis=-1, keepdims=True)
    order = np.argsort(-sc, axis=-1, kind="stable")
    idx = order[:, :TOPK]  # [N, 2]
    s = np.take_along_axis(sc, idx, axis=-1)  # [N, 2]

    usage = sc.mean(axis=0)
    lb_loss = np.float32(-(usage * np.log(usage + 1e-9)).sum())

    # ---- Dispatch: gather each expert's tokens
    tok_lists = []
    wt_lists = []
    for e in range(E):
        toks = []
        wts = []
        for k in range(TOPK):
            m = idx[:, k] == e
            toks.append(np.nonzero(m)[0])
            wts.append(s[m, k])
        tok_lists.append(np.concatenate(toks))
        wt_lists.append(np.concatenate(wts).astype(np.float32))

    cmax = max(len(t) for t in tok_lists)
    C = max(256, -(-cmax // 256) * 256)
    blocks = []
    off = 0
    while off < C:
        bs = 512 if C - off >= 512 else 256
        blocks.append((off, bs))
        off += bs

    use_b12 = bool(np.any(b1) or np.any(b2))
    nc = _get_nc(blocks, C, use_b12)

    in_maps = []
    for e in range(E):
        toks = tok_lists[e]
        ce = len(toks)
        xTg = np.zeros((DIM, C), dtype=np.float32)
        xTg[:, :ce] = xf[toks].T
        wtok = np.zeros((C,), dtype=np.float32)
        wtok[:ce] = wt_lists[e] * np.float32(INV_SQRT2)
        m = {
            "xT": xTg,
            "w1": np.ascontiguousarray(w1[e]),
            "w2": np.ascontiguousarray(w2[e]),
            "w3": np.ascontiguousarray(w3[e]),
            "wtok": np.ascontiguousarray(np.broadcast_to(wtok, (P, C))),
        }
        if use_b12:
            m["b1"] = np.ascontiguousarray(b1[e])
            m["b2"] = np.ascontiguousarray(b2[e])
        in_maps.append(m)

    res = run_bass_kernel_spmd(nc, in_maps, list(range(E))).results

    # ---- Combine: out[t] = sum over t's two experts of weighted outputs
    outT = np.zeros((DIM, N), dtype=np.float32)
    for e in range(E):
        toks = tok_lists[e]
        outT[:, toks] += res[e]["y"][:, : len(toks)]

    if np.any(b3):
        bsel = (
            b3[idx[:, 0]] * (s[:, 0:1] * INV_SQRT2)
            + b3[idx[:, 1]] * (s[:, 1:2] * INV_SQRT2)
        ).astype(np.float32)  # [N, DIM]
        outT += bsel.T

    out = np.ascontiguousarray(outT.T).reshape(B, S, D)
    return out, lb_loss


# revision 15
# speedup vs baseline: 1.2186x; 1.2186x over previous
"""MoE layer (8 experts, top-2, SwiGLU) on 8 Trainium2 NeuronCores.

Strategy (expert-parallel, per the sharding hint):
  - Host computes the tiny gate (0.07% of FLOPs) in float64 — this is the
    routing/dispatch metadata plus the load-balance loss scalar.
  - Tokens are dispatched to their top-2 experts; expert e's token set is
    gathered, transposed to [DIM, C] and shipped to core e.
  - Each core runs the full SwiGLU FFN for its expert in float32r (full-rate
    fp32 on the PE array) over its gathered tokens:
      phase A: hT = w1^T xT ; gT = silu(w2^T xT) ; hgT = hT*gT -> DRAM stage
      phase B: yT = w3^T hgT, scaled per-token by the combine weight.
  - Host scatter-adds the two weighted expert contributions per token.

All activations stay transposed ([feature, token]) so every matmul uses the
weights in their natural layout and no on-device transposes are needed.
"""

import os
import sys

for _p in ("/opt/trn_rl_repo", "/root/.axon_site/_ro/trn_rl_repo"):
    if os.path.isdir(_p) and _p not in sys.path:
        sys.path.insert(0, _p)

import numpy as np

DIM = 1024
HID = 2048
E = 8
TOPK = 2
INV_SQRT2 = 1.0 / 1.41421356237
P = 128
KO_D = DIM // P  # 8 k-subtiles for DIM contraction
KO_H = HID // P  # 16 k-subtiles for HID contraction
JCH = 4  # hidden chunks in phase A (HID/JCH = 512 wide)
HC = HID // JCH  # 512
MI_N = HC // P  # 4 psum m-subtiles per hidden chunk

_cache = {}


def _build_nc(blocks, C, use_b12):
    import concourse.mybir as mybir
    import concourse.tile as tile
    from concourse import bacc

    R = mybir.dt.float32r
    F = mybir.dt.float32

    nc = bacc.Bacc("TRN2", target_bir_lowering=False, debug=False, num_devices=E)

    xT = nc.dram_tensor("xT", [DIM, C], R, kind="ExternalInput")
    w1 = nc.dram_tensor("w1", [DIM, HID], R, kind="ExternalInput")
    w2 = nc.dram_tensor("w2", [DIM, HID], R, kind="ExternalInput")
    w3 = nc.dram_tensor("w3", [HID, DIM], R, kind="ExternalInput")
    wtok = nc.dram_tensor("wtok", [P, C], F, kind="ExternalInput")
    if use_b12:
        b1d = nc.dram_tensor("b1", [HID], F, kind="ExternalInput")
        b2d = nc.dram_tensor("b2", [HID], F, kind="ExternalInput")
    y = nc.dram_tensor("y", [DIM, C], F, kind="ExternalOutput")

    xT_t = xT.ap().rearrange("(ko p) c -> p ko c", p=P)
    w1_t = w1.ap().rearrange("(ko p) h -> p ko h", p=P)
    w2_t = w2.ap().rearrange("(ko p) h -> p ko h", p=P)
    w3_t = w3.ap().rearrange("(ko p) d -> p ko d", p=P)
    y_t = y.ap().rearrange("(mo p) c -> p mo c", p=P)

    W3PRE = 8  # w3 k-chunks prefetched into spare SBUF during phase A
    HGPRE = 8  # hg block-0 k-chunks prefetched during phase A

    with tile.TileContext(nc) as tc:
        with (
            tc.tile_pool(name="dram", bufs=1, space="DRAM") as drampool,
            tc.tile_pool(name="w3pre", bufs=1) as w3pre,
            tc.tile_pool(name="hgpre", bufs=1) as hgpre,
        ):
            hg_stage = drampool.tile([P, KO_H, C], R)

            w3_sb = [None] * KO_H
            hgb0_pre = [None] * KO_H

            # ---------------- Phase A: hgT = (w1^T x^T) * silu(w2^T x^T)
            # j-outer / block-inner: x fully resident (loaded once),
            # w1/w2 streamed per hidden chunk (double-buffered tags).
            with (
                tc.tile_pool(name="wA", bufs=2) as wA,
                tc.tile_pool(name="xA", bufs=1) as xA,
                tc.tile_pool(name="workA", bufs=3) as workA,
                tc.tile_pool(name="psA", bufs=1, space="PSUM") as psA,
            ):
                def load_xblock(bi, c0, bs):
                    tiles = []
                    for ko in range(KO_D):
                        t = xA.tile(
                            [P, bs], R, tag=f"x{ko}b{bi}", name=f"x{ko}b{bi}"
                        )
                        nc.sync.dma_start(t[:], xT_t[:, ko, c0 : c0 + bs])
                        tiles.append(t)
                    return tiles

                def load_wj(j):
                    t1s, t2s = [], []
                    for ko in range(KO_D):
                        t1 = wA.tile([P, HC], R, tag=f"w1k{ko}", name=f"w1k{ko}")
                        nc.sync.dma_start(t1[:], w1_t[:, ko, j * HC : (j + 1) * HC])
                        t2 = wA.tile([P, HC], R, tag=f"w2k{ko}", name=f"w2k{ko}")
                        nc.sync.dma_start(t2[:], w2_t[:, ko, j * HC : (j + 1) * HC])
                        t1s.append(t1)
                        t2s.append(t2)
                    return t1s, t2s

                # Emission order = DMA priority: block-0 x, j0 weights,
                # remaining x blocks, later weights on demand in the j loop.
                xtiles = [None] * len(blocks)
                xtiles[0] = load_xblock(0, *blocks[0])
                wj_next = load_wj(0)
                for bi_ in range(1, len(blocks)):
                    xtiles[bi_] = load_xblock(bi_, *blocks[bi_])

                if use_b12:
                    b1_sb = xA.tile([P, KO_H], F, tag="b1")
                    b2_sb = xA.tile([P, KO_H], F, tag="b2")
                    nc.sync.dma_start(
                        b1_sb[:], b1d.ap().rearrange("(ko p) -> p ko", p=P)
                    )
                    nc.sync.dma_start(
                        b2_sb[:], b2d.ap().rearrange("(ko p) -> p ko", p=P)
                    )

                for j in range(JCH):
                    w1_sb, w2_sb = wj_next
                    if j + 1 < JCH:
                        wj_next = load_wj(j + 1)
                    for bi_, (c0, bs) in enumerate(blocks):
                        xb = xtiles[bi_]
                        for mi in range(MI_N):
                            hs = slice(mi * P, (mi + 1) * P)
                            h_ps = psA.tile([P, 512], F, tag=f"h{mi}")
                            g_ps = psA.tile([P, 512], F, tag=f"g{mi}")
                            for ki in range(KO_D):
                                nc.tensor.matmul(
                                    h_ps[:, :bs],
                                    w1_sb[ki][:, hs],
                                    xb[ki][:],
                                    start=(ki == 0),
                                    stop=(ki == KO_D - 1),
                                )
                            for ki in range(KO_D):
                                nc.tensor.matmul(
                                    g_ps[:, :bs],
                                    w2_sb[ki][:, hs],
                                    xb[ki][:],
                                    start=(ki == 0),
                                    stop=(ki == KO_D - 1),
                                )
                            hidx = j * MI_N + mi
                            gact = workA.tile([P, 512], F, tag="gact")
                            if use_b12:
                                nc.scalar.activation(
                                    gact[:, :bs],
                                    g_ps[:, :bs],
                                    mybir.ActivationFunctionType.Silu,
                                    bias=b2_sb[:, hidx : hidx + 1],
                                )
                                nc.vector.tensor_scalar_add(
                                    h_ps[:, :bs],
                                    h_ps[:, :bs],
                                    b1_sb[:, hidx : hidx + 1],
                                )
                            else:
                                nc.scalar.activation(
                                    gact[:, :bs],
                                    g_ps[:, :bs],
                                    mybir.ActivationFunctionType.Silu,
                                )
                            hg = workA.tile([P, 512], R, tag="hg")
                            nc.vector.tensor_mul(
                                hg[:, :bs], h_ps[:, :bs], gact[:, :bs]
                            )
                            nc.sync.dma_start(
                                hg_stage[:, hidx, c0 : c0 + bs], hg[:, :bs]
                            )

                # Prefetch w3 first half + hg block-0 first half into spare
                # SBUF while phase A drains (emitted last = low priority).
                for ko in range(W3PRE):
                    t = w3pre.tile([P, DIM], R, tag=f"w3k{ko}", name=f"w3k{ko}")
                    nc.sync.dma_start(t[:], w3_t[:, ko, :])
                    w3_sb[ko] = t
                bs0 = blocks[0][1]
                for ko in range(HGPRE):
                    t = hgpre.tile([P, bs0], R, tag=f"hgp{ko}", name=f"hgp{ko}")
                    nc.sync.dma_start(t[:], hg_stage[:, ko, blocks[0][0] : blocks[0][0] + bs0])
                    hgb0_pre[ko] = t

            # ---------------- Phase B: yT = w3^T hgT, scaled by wtok
            with (
                tc.tile_pool(name="wB", bufs=1) as wB,
                tc.tile_pool(name="hgB", bufs=3) as hgB,
                tc.tile_pool(name="outB", bufs=4) as outB,
                tc.tile_pool(name="psB", bufs=1, space="PSUM") as psB,
            ):
                wtok_sb = wB.tile([P, C], F, tag="wtok")
                nc.sync.dma_start(wtok_sb[:], wtok.ap())
                for ko in range(W3PRE, KO_H):
                    t = wB.tile([P, DIM], R, tag=f"w3k{ko}", name=f"w3k{ko}")
                    nc.sync.dma_start(t[:], w3_t[:, ko, :])
                    w3_sb[ko] = t

                MO_N = DIM // P
                for bi_, (c0, bs) in enumerate(blocks):
                    hgb = []
                    for ko in range(KO_H):
                        if bi_ == 0 and ko < HGPRE:
                            hgb.append(hgb0_pre[ko])
                            continue
                        t = hgB.tile([P, 512], R, tag=f"hgb{ko}", name=f"hgb{ko}")
                        nc.sync.dma_start(
                            t[:, :bs], hg_stage[:, ko, c0 : c0 + bs]
                        )
                        hgb.append(t)
                    # Split-k groups: banks dwell for 8 consecutive matmuls
                    # (avoids per-MM PSUM bank cycling) while compute can
                    # start on the prefetched first half.
                    y_ps = [
                        psB.tile([P, 512], F, tag=f"y{mo}", name=f"y{mo}")
                        for mo in range(MO_N)
                    ]
                    for k0 in range(0, KO_H, 8):
                        for mo in range(MO_N):
                            for ki in range(k0, k0 + 8):
                                nc.tensor.matmul(
                                    y_ps[mo][:, :bs],
                                    w3_sb[ki][:, mo * P : (mo + 1) * P],
                                    hgb[ki][:, :bs],
                                    start=(ki == 0),
                                    stop=(ki == KO_H - 1),
                                )
                    for mo in range(MO_N):
                        y_sb = outB.tile([P, 512], F, tag="ysb")
                        nc.vector.tensor_mul(
                            y_sb[:, :bs], y_ps[mo][:, :bs], wtok_sb[:, c0 : c0 + bs]
                        )
                        nc.sync.dma_start(y_t[:, mo, c0 : c0 + bs], y_sb[:, :bs])

    nc.compile()
    return nc


def _get_nc(blocks, C, use_b12):
    key = (tuple(blocks), C, use_b12)
    if key not in _cache:
        _cache[key] = _build_nc(blocks, C, use_b12)
    return _cache[key]


def kernel(x, gate_w, gate_b, w1, b1, w2, b2, w3, b3):
    from concourse.bass_utils import run_bass_kernel_spmd

    x = np.asarray(x, dtype=np.float32)
    gate_w = np.asarray(gate_w, dtype=np.float32)
    gate_b = np.asarray(gate_b, dtype=np.float32)
    w1 = np.asarray(w1, dtype=np.float32)
    b1 = np.asarray(b1, dtype=np.float32)
    w2 = np.asarray(w2, dtype=np.float32)
    b2 = np.asarray(b2, dtype=np.float32)
    w3 = np.asarray(w3, dtype=np.float32)
    b3 = np.asarray(b3, dtype=np.float32)

    B, S, D = x.shape
    N = B * S
    xf = x.reshape(N, D)

    # ---- Gating / routing metadata (float64 so top-k decisions are exact)
    logits = xf.astype(np.float64) @ gate_w.astype(np.float64) + gate_b
    logits -= logits.max(axis=-1, keepdims=True)
    sc = np.exp(logits)
    sc /= sc.sum(axis=-1, keepdims=True)
    order = np.argsort(-sc, axis=-1, kind="stable")
    idx = order[:, :TOPK]  # [N, 2]
    s = np.take_along_axis(sc, idx, axis=-1)  # [N, 2]

    usage = sc.mean(axis=0)
    lb_loss = np.float32(-(usage * np.log(usage + 1e-9)).sum())

    # ---- Dispatch: gather each expert's tokens
    tok_lists = []
    wt_lists = []
    for e in range(E):
        toks = []
        wts = []
        for k in range(TOPK):
            m = idx[:, k] == e
            toks.append(np.nonzero(m)[0])
            wts.append(s[m, k])
        tok_lists.append(np.concatenate(toks))
        wt_lists.append(np.concatenate(wts).astype(np.float32))

    cmax = max(len(t) for t in tok_lists)
    C = max(256, -(-cmax // 256) * 256)
    blocks = []
    off = 0
    while off < C:
        bs = 512 if C - off >= 512 else 256
        blocks.append((off, bs))
        off += bs

    use_b12 = bool(np.any(b1) or np.any(b2))
    nc = _get_nc(blocks, C, use_b12)

    in_maps = []
    for e in range(E):
        toks = tok_lists[e]
        ce = len(toks)
        xTg = np.zeros((DIM, C), dtype=np.float32)
        xTg[:, :ce] = xf[toks].T
        wtok = np.zeros((C,), dtype=np.float32)
        wtok[:ce] = wt_lists[e] * np.float32(INV_SQRT2)
        m = {
            "xT": xTg,
            "w1": np.ascontiguousarray(w1[e]),
            "w2": np.ascontiguousarray(w2[e]),
            "w3": np.ascontiguousarray(w3[e]),
            "wtok": np.ascontiguousarray(np.broadcast_to(wtok, (P, C))),
        }
        if use_b12:
            m["b1"] = np.ascontiguousarray(b1[e])
            m["b2"] = np.ascontiguousarray(b2[e])
        in_maps.append(m)

    res = run_bass_kernel_spmd(nc, in_maps, list(range(E))).results

    # ---- Combine: out[t] = sum over t's two experts of weighted outputs
    outT = np.zeros((DIM, N), dtype=np.float32)
    for e in range(E):
        toks = tok_lists[e]
        outT[:, toks] += res[e]["y"][:, : len(toks)]

    if np.any(b3):
        bsel = (
            b3[idx[:, 0]] * (s[:, 0:1] * INV_SQRT2)
            + b3[idx[:, 1]] * (s[:, 1:2] * INV_SQRT2)
        ).astype(np.float32)  # [N, DIM]
        outT += bsel.T

    out = np.ascontiguousarray(outT.T).reshape(B, S, D)
    return out, lb_loss


# revision 29
# speedup vs baseline: 1.3000x; 1.0668x over previous
"""MoE layer (8 experts, top-2, SwiGLU) on 8 Trainium2 NeuronCores.

Strategy (expert-parallel, per the sharding hint):
  - Host computes the tiny gate (0.07% of FLOPs) in float64 — this is the
    routing/dispatch metadata plus the load-balance loss scalar.
  - Tokens are dispatched to their top-2 experts; expert e's token set is
    gathered, transposed to [DIM, C] and shipped to core e.
  - Each core runs the full SwiGLU FFN for its expert in float32r (full-rate
    fp32 on the PE array) over its gathered tokens:
      phase A: hT = w1^T xT ; gT = silu(w2^T xT) ; hgT = hT*gT -> DRAM stage
      phase B: yT = w3^T hgT, scaled per-token by the combine weight.
  - Host scatter-adds the two weighted expert contributions per token.

All activations stay transposed ([feature, token]) so every matmul uses the
weights in their natural layout and no on-device transposes are needed.
"""

import os
import sys

for _p in ("/opt/trn_rl_repo", "/root/.axon_site/_ro/trn_rl_repo"):
    if os.path.isdir(_p) and _p not in sys.path:
        sys.path.insert(0, _p)

import numpy as np

DIM = 1024
HID = 2048
E = 8
TOPK = 2
INV_SQRT2 = 1.0 / 1.41421356237
P = 128
KO_D = DIM // P  # 8 k-subtiles for DIM contraction
KO_H = HID // P  # 16 k-subtiles for HID contraction
JCH = 4  # hidden chunks in phase A (HID/JCH = 512 wide)
HC = HID // JCH  # 512
MI_N = HC // P  # 4 psum m-subtiles per hidden chunk

_cache = {}


def _build_nc(blocks, C, use_b12):
    import concourse.mybir as mybir
    import concourse.tile as tile
    from concourse import bacc

    R = mybir.dt.float32r
    F = mybir.dt.float32

    nc = bacc.Bacc("TRN2", target_bir_lowering=False, debug=False, num_devices=E)

    xT = nc.dram_tensor("xT", [DIM, C], R, kind="ExternalInput")
    w1 = nc.dram_tensor("w1", [DIM, HID], R, kind="ExternalInput")
    w2 = nc.dram_tensor("w2", [DIM, HID], R, kind="ExternalInput")
    w3 = nc.dram_tensor("w3", [HID, DIM], R, kind="ExternalInput")
    wtok = nc.dram_tensor("wtok", [P, C], F, kind="ExternalInput")
    if use_b12:
        b1d = nc.dram_tensor("b1", [HID], F, kind="ExternalInput")
        b2d = nc.dram_tensor("b2", [HID], F, kind="ExternalInput")
    y = nc.dram_tensor("y", [DIM, C], F, kind="ExternalOutput")

    xT_t = xT.ap().rearrange("(ko p) c -> p ko c", p=P)
    w1_t = w1.ap().rearrange("(ko p) h -> p ko h", p=P)
    w2_t = w2.ap().rearrange("(ko p) h -> p ko h", p=P)
    w3_t = w3.ap().rearrange("(ko p) d -> p ko d", p=P)
    y_t = y.ap().rearrange("(mo p) c -> p mo c", p=P)

    W3PRE = 8  # w3 k-chunks prefetched into spare SBUF during phase A
    HGPRE = 8  # hg block-0 k-chunks prefetched during phase A

    with tile.TileContext(nc) as tc:
        with (
            tc.tile_pool(name="dram", bufs=1, space="DRAM") as drampool,
            tc.tile_pool(name="w3pre", bufs=1) as w3pre,
            tc.tile_pool(name="hgpre", bufs=1) as hgpre,
        ):
            hg_stage = drampool.tile([P, KO_H, C], R)

            w3_sb = [None] * KO_H
            hgb0_pre = [None] * KO_H

            # ---------------- Phase A: hgT = (w1^T x^T) * silu(w2^T x^T)
            # j-outer / block-inner: x fully resident (loaded once),
            # w1/w2 streamed per hidden chunk (double-buffered tags).
            with (
                tc.tile_pool(name="wA", bufs=2) as wA,
                tc.tile_pool(name="xA", bufs=1) as xA,
                tc.tile_pool(name="workA", bufs=3) as workA,
                tc.tile_pool(name="psA", bufs=1, space="PSUM") as psA,
            ):
                def load_xblock(bi, c0, bs):
                    # Alternate gpsimd/sync DGE streams: parallel issue during
                    # the bandwidth-critical startup, and no single queue
                    # serializes all x loads.
                    tiles = []
                    for ko in range(KO_D):
                        t = xA.tile(
                            [P, bs], R, tag=f"x{ko}b{bi}", name=f"x{ko}b{bi}"
                        )
                        eng = nc.gpsimd if (ko + bi) % 2 == 0 else nc.sync
                        eng.dma_start(t[:], xT_t[:, ko, c0 : c0 + bs])
                        tiles.append(t)
                    return tiles

                def load_wj(j):
                    t1s, t2s = [], []
                    for ko in range(KO_D):
                        t1 = wA.tile([P, HC], R, tag=f"w1k{ko}", name=f"w1k{ko}")
                        nc.sync.dma_start(t1[:], w1_t[:, ko, j * HC : (j + 1) * HC])
                        t2 = wA.tile([P, HC], R, tag=f"w2k{ko}", name=f"w2k{ko}")
                        nc.sync.dma_start(t2[:], w2_t[:, ko, j * HC : (j + 1) * HC])
                        t1s.append(t1)
                        t2s.append(t2)
                    return t1s, t2s

                # PE warm-up: the first ~10us are DMA-latency idle; run dummy
                # matmuls on a zeroed tile so the HAM clock-gate opens before
                # real work arrives (cold MMs run at half clock otherwise).
                warm_sb = workA.tile([P, 512], F, tag="warm", bufs=1)
                nc.vector.memset(warm_sb[:], 0.0)
                for _w in range(3):
                    warm_ps = psA.tile([P, 512], F, tag="h0")
                    nc.tensor.matmul(
                        warm_ps[:], warm_sb[:, :P], warm_sb[:], start=True, stop=True
                    )

                # Emission order = DMA priority: block-0 x interleaved with j0
                # weights per k-chunk (fastest warm-up of the first ki
                # pipeline), then remaining x blocks, later weights streamed
                # on demand in the j loop.
                xtiles = [None] * len(blocks)
                c0_0, bs_0 = blocks[0]
                xb0 = []
                w1j0, w2j0 = [], []
                for ko in range(KO_D):
                    tx = xA.tile([P, bs_0], R, tag=f"x{ko}b0", name=f"x{ko}b0")
                    nc.sync.dma_start(tx[:], xT_t[:, ko, c0_0 : c0_0 + bs_0])
                    xb0.append(tx)
                    t1 = wA.tile([P, HC], R, tag=f"w1k{ko}", name=f"w1k{ko}")
                    nc.sync.dma_start(t1[:], w1_t[:, ko, 0:HC])
                    w1j0.append(t1)
                    t2 = wA.tile([P, HC], R, tag=f"w2k{ko}", name=f"w2k{ko}")
                    nc.sync.dma_start(t2[:], w2_t[:, ko, 0:HC])
                    w2j0.append(t2)
                xtiles[0] = xb0
                wj_next = (w1j0, w2j0)
                for bi_ in range(1, len(blocks)):
                    xtiles[bi_] = load_xblock(bi_, *blocks[bi_])

                if use_b12:
                    b1_sb = xA.tile([P, KO_H], F, tag="b1")
                    b2_sb = xA.tile([P, KO_H], F, tag="b2")
                    nc.sync.dma_start(
                        b1_sb[:], b1d.ap().rearrange("(ko p) -> p ko", p=P)
                    )
                    nc.sync.dma_start(
                        b2_sb[:], b2d.ap().rearrange("(ko p) -> p ko", p=P)
                    )

                for j in range(JCH):
                    w1_sb, w2_sb = wj_next
                    if j + 1 < JCH:
                        wj_next = load_wj(j + 1)
                    for bi_, (c0, bs) in enumerate(blocks):
                        xb = xtiles[bi_]
                        for mi in range(MI_N):
                            hs = slice(mi * P, (mi + 1) * P)
                            h_ps = psA.tile([P, 512], F, tag=f"h{mi}")
                            g_ps = psA.tile([P, 512], F, tag=f"g{mi}")
                            for ki in range(KO_D):
                                nc.tensor.matmul(
                                    h_ps[:, :bs],
                                    w1_sb[ki][:, hs],
                                    xb[ki][:],
                                    start=(ki == 0),
                                    stop=(ki == KO_D - 1),
                                )
                            for ki in range(KO_D):
                                nc.tensor.matmul(
                                    g_ps[:, :bs],
                                    w2_sb[ki][:, hs],
                                    xb[ki][:],
                                    start=(ki == 0),
                                    stop=(ki == KO_D - 1),
                                )
                            hidx = j * MI_N + mi
                            gact = workA.tile([P, 512], F, tag="gact")
                            if use_b12:
                                nc.scalar.activation(
                                    gact[:, :bs],
                                    g_ps[:, :bs],
                                    mybir.ActivationFunctionType.Silu,
                                    bias=b2_sb[:, hidx : hidx + 1],
                                )
                                nc.vector.tensor_scalar_add(
                                    h_ps[:, :bs],
                                    h_ps[:, :bs],
                                    b1_sb[:, hidx : hidx + 1],
                                )
                            else:
                                nc.scalar.activation(
                                    gact[:, :bs],
                                    g_ps[:, :bs],
                                    mybir.ActivationFunctionType.Silu,
                                )
                            hg = workA.tile([P, 512], R, tag="hg")
                            nc.vector.tensor_mul(
                                hg[:, :bs], h_ps[:, :bs], gact[:, :bs]
                            )
                            nc.sync.dma_start(
                                hg_stage[:, hidx, c0 : c0 + bs], hg[:, :bs]
                            )

                # Prefetch w3 first half + hg block-0 first half into spare
                # SBUF while phase A drains (emitted last = low priority).
                for ko in range(W3PRE):
                    t = w3pre.tile([P, DIM], R, tag=f"w3k{ko}", name=f"w3k{ko}")
                    nc.sync.dma_start(t[:], w3_t[:, ko, :])
                    w3_sb[ko] = t
                bs0 = blocks[0][1]
                for ko in range(HGPRE):
                    t = hgpre.tile([P, bs0], R, tag=f"hgp{ko}", name=f"hgp{ko}")
                    nc.sync.dma_start(t[:], hg_stage[:, ko, blocks[0][0] : blocks[0][0] + bs0])
                    hgb0_pre[ko] = t

            # ---------------- Phase B: yT = w3^T hgT, scaled by wtok
            with (
                tc.tile_pool(name="wB", bufs=1) as wB,
                tc.tile_pool(name="hgB", bufs=3) as hgB,
                tc.tile_pool(name="outB", bufs=4) as outB,
                tc.tile_pool(name="psB", bufs=1, space="PSUM") as psB,
            ):
                wtok_sb = wB.tile([P, C], F, tag="wtok")
                nc.sync.dma_start(wtok_sb[:], wtok.ap())
                for ko in range(W3PRE, KO_H):
                    t = wB.tile([P, DIM], R, tag=f"w3k{ko}", name=f"w3k{ko}")
                    nc.sync.dma_start(t[:], w3_t[:, ko, :])
                    w3_sb[ko] = t

                MO_N = DIM // P

                def load_hgblock(bi):
                    c0, bs = blocks[bi]
                    tiles = []
                    for ko in range(KO_H):
                        if bi == 0 and ko < HGPRE:
                            tiles.append(hgb0_pre[ko])
                            continue
                        t = hgB.tile([P, 512], R, tag=f"hgb{ko}", name=f"hgb{ko}")
                        nc.sync.dma_start(
                            t[:, :bs], hg_stage[:, ko, c0 : c0 + bs]
                        )
                        tiles.append(t)
                    return tiles

                NB = len(blocks)
                hgb_q = [load_hgblock(b) for b in range(min(2, NB))]
                for bi_, (c0, bs) in enumerate(blocks):
                    # Prefetch block bi_+2 before this block's y writes are
                    # emitted, so hg reads never queue behind y writes.
                    if bi_ + 2 < NB:
                        hgb_q.append(load_hgblock(bi_ + 2))
                    hgb = hgb_q[bi_]
                    # Split-k groups: banks dwell for 8 consecutive matmuls
                    # (avoids per-MM PSUM bank cycling) while compute can
                    # start on the prefetched first half.
                    y_ps = [
                        psB.tile([P, 512], F, tag=f"y{mo}", name=f"y{mo}")
                        for mo in range(MO_N)
                    ]
                    for k0 in range(0, KO_H, 8):
                        for mo in range(MO_N):
                            for ki in range(k0, k0 + 8):
                                nc.tensor.matmul(
                                    y_ps[mo][:, :bs],
                                    w3_sb[ki][:, mo * P : (mo + 1) * P],
                                    hgb[ki][:, :bs],
                                    start=(ki == 0),
                                    stop=(ki == KO_H - 1),
                                )
                    for mo in range(MO_N):
                        y_sb = outB.tile([P, 512], F, tag="ysb")
                        nc.vector.tensor_mul(
                            y_sb[:, :bs], y_ps[mo][:, :bs], wtok_sb[:, c0 : c0 + bs]
                        )
                        nc.sync.dma_start(y_t[:, mo, c0 : c0 + bs], y_sb[:, :bs])

    nc.compile()
    return nc


def _get_nc(blocks, C, use_b12):
    key = (tuple(blocks), C, use_b12)
    if key not in _cache:
        _cache[key] = _build_nc(blocks, C, use_b12)
    return _cache[key]


def kernel(x, gate_w, gate_b, w1, b1, w2, b2, w3, b3):
    from concourse.bass_utils import run_bass_kernel_spmd

    x = np.asarray(x, dtype=np.float32)
    gate_w = np.asarray(gate_w, dtype=np.float32)
    gate_b = np.asarray(gate_b, dtype=np.float32)
    w1 = np.asarray(w1, dtype=np.float32)
    b1 = np.asarray(b1, dtype=np.float32)
    w2 = np.asarray(w2, dtype=np.float32)
    b2 = np.asarray(b2, dtype=np.float32)
    w3 = np.asarray(w3, dtype=np.float32)
    b3 = np.asarray(b3, dtype=np.float32)

    B, S, D = x.shape
    N = B * S
    xf = x.reshape(N, D)

    # ---- Gating / routing metadata (float64 so top-k decisions are exact)
    logits = xf.astype(np.float64) @ gate_w.astype(np.float64) + gate_b
    logits -= logits.max(axis=-1, keepdims=True)
    sc = np.exp(logits)
    sc /= sc.sum(axis=-1, keepdims=True)
    order = np.argsort(-sc, axis=-1, kind="stable")
    idx = order[:, :TOPK]  # [N, 2]
    s = np.take_along_axis(sc, idx, axis=-1)  # [N, 2]

    usage = sc.mean(axis=0)
    lb_loss = np.float32(-(usage * np.log(usage + 1e-9)).sum())

    # ---- Dispatch: gather each expert's tokens
    tok_lists = []
    wt_lists = []
    for e in range(E):
        toks = []
        wts = []
        for k in range(TOPK):
            m = idx[:, k] == e
            toks.append(np.nonzero(m)[0])
            wts.append(s[m, k])
        tok_lists.append(np.concatenate(toks))
        wt_lists.append(np.concatenate(wts).astype(np.float32))

    # Capacity: any block >= 256 wide runs fp32r matmuls at full rate, so pad
    # only to a multiple of 32 (DMA-friendly) with a >=256 tail split.
    cmax = max(len(t) for t in tok_lists)
    C = max(256, -(-cmax // 32) * 32)
    blocks = []
    off = 0
    rem = C
    while rem > 0:
        if rem > 512 + 256:
            bs = 512
        elif rem > 512:
            bs = rem - 256  # in (256, 512]
        else:
            bs = rem  # in [256, 512]
        blocks.append((off, bs))
        off += bs
        rem -= bs

    use_b12 = bool(np.any(b1) or np.any(b2))
    nc = _get_nc(blocks, C, use_b12)

    in_maps = []
    for e in range(E):
        toks = tok_lists[e]
        ce = len(toks)
        xTg = np.zeros((DIM, C), dtype=np.float32)
        xTg[:, :ce] = xf[toks].T
        wtok = np.zeros((C,), dtype=np.float32)
        wtok[:ce] = wt_lists[e] * np.float32(INV_SQRT2)
        m = {
            "xT": xTg,
            "w1": np.ascontiguousarray(w1[e]),
            "w2": np.ascontiguousarray(w2[e]),
            "w3": np.ascontiguousarray(w3[e]),
            "wtok": np.ascontiguousarray(np.broadcast_to(wtok, (P, C))),
        }
        if use_b12:
            m["b1"] = np.ascontiguousarray(b1[e])
            m["b2"] = np.ascontiguousarray(b2[e])
        in_maps.append(m)

    try:
        res = run_bass_kernel_spmd(nc, in_maps, list(range(E))).results
    except Exception:
        # Transient device errors (e.g. a wedged core from a prior run)
        # usually clear on retry.
        import time as _time

        _time.sleep(2.0)
        res = run_bass_kernel_spmd(nc, in_maps, list(range(E))).results

    # ---- Combine: out[t] = sum over t's two experts of weighted outputs
    outT = np.zeros((DIM, N), dtype=np.float32)
    for e in range(E):
        toks = tok_lists[e]
        outT[:, toks] += res[e]["y"][:, : len(toks)]

    if np.any(b3):
        bsel = (
            b3[idx[:, 0]] * (s[:, 0:1] * INV_SQRT2)
            + b3[idx[:, 1]] * (s[:, 1:2] * INV_SQRT2)
        ).astype(np.float32)  # [N, DIM]
        outT += bsel.T

    out = np.ascontiguousarray(outT.T).reshape(B, S, D)
    return out, lb_loss
